# revision 7
# baseline (speedup 1.0000x reference)
# Trainium2 Bass kernel for nn_Attention3 (unnormalized linear attention).
#
# Math: e_i = x @ W_i.T + b_i (i=1,2,3);  out = sigmoid((e1 @ e2.T @ e3) @ WO.T + bO)
# Since there is no softmax, (e1 @ e2.T) @ e3 == e1 @ (e2.T @ e3) where
# KV = e2.T @ e3 is only [64, 64].  The kernel is therefore memory-bound:
# read x once, write out once.
#
# Sharding: the flattened [B*S, 512] = [16384, 512] rows are split into 8
# contiguous chunks of 2048 rows (cores 0-3 <- batch 0, cores 4-7 <- batch 1).
# Each core computes its partial KV^T = e3_c.T @ e2_c over its rows, all 8
# cores AllGather the partials and each sums the four belonging to its batch
# (selected by a per-core 0/1 mask input), then each core finishes
# out = sigmoid(e1 @ (KV @ WO.T) + bO) for its rows.
#
# Precision/layout: x arrives host-transposed ([512, rows] f32) so the
# contraction dim sits on SBUF partitions, and is cast on-chip to float32r
# (tf32-grade, full PE rate at free-dim >= 256).  Projections are computed
# transposed ([64|128, 512] out tiles, N=512) in f32r; e2|e3 is bias-added and
# PE-transposed back to natural layout in full f32 for the KV^T accumulation.
# The final e1 @ (KV @ WO.T) matmul runs in f32r at N=512.

import numpy as np

import concourse.mybir as mybir
import concourse.tile as tile
from concourse import bacc
from concourse.bass_utils import run_bass_kernel_spmd

BATCH = 2
SEQ = 8192
DIN = 512
DE = 64
N_CORES = 8
ROWS = (BATCH * SEQ) // N_CORES  # 2048 rows per core

TRACE = False
TRACE_KWARGS = {}
LAST_RESULT = None

_NC_CACHE = {}


def build_nc(rows=ROWS, n_cores=N_CORES, collective="ag8"):
    f32 = mybir.dt.float32
    f32r = mybir.dt.float32r
    add = mybir.AluOpType.add
    mult = mybir.AluOpType.mult

    assert rows % 512 == 0
    n_chunks = rows // 512

    nc = bacc.Bacc(None, target_bir_lowering=False, debug=False, num_devices=n_cores)

    xt = nc.dram_tensor("xt", [DIN, rows], f32, kind="ExternalInput")
    w1t = nc.dram_tensor("w1t", [DIN, DE], f32, kind="ExternalInput")
    w23t = nc.dram_tensor("w23t", [DIN, 2 * DE], f32, kind="ExternalInput")
    wot = nc.dram_tensor("wot", [DE, DIN], f32, kind="ExternalInput")
    b1 = nc.dram_tensor("b1", [DE, 1], f32, kind="ExternalInput")
    b23 = nc.dram_tensor("b23", [2 * DE, 1], f32, kind="ExternalInput")
    bo = nc.dram_tensor("bo", [1, DIN], f32, kind="ExternalInput")
    ident = nc.dram_tensor("ident", [128, 128], f32, kind="ExternalInput")
    kvmask = nc.dram_tensor("kvmask", [1, n_cores], f32, kind="ExternalInput")
    out = nc.dram_tensor("out", [rows, DIN], f32, kind="ExternalOutput")

    xt_t = xt.ap().rearrange("(kt p) s -> p kt s", p=128)  # [128, 4, rows]

    with tile.TileContext(nc) as tc:
        with (
            tc.tile_pool(name="consts", bufs=1) as consts,
            tc.tile_pool(name="persist", bufs=1) as persist,
            tc.tile_pool(name="kvps", bufs=1, space="PSUM") as kvps,
            tc.tile_pool(name="dram", bufs=1, space="DRAM") as dram,
        ):
            # constants: HWDGE f32 loads, DVE-rounded to f32r where needed
            sb_w1t_f = consts.tile([128, 4, DE], f32)
            nc.sync.dma_start(out=sb_w1t_f, in_=w1t.ap().rearrange("(kt p) d -> p kt d", p=128))
            sb_w1t = consts.tile([128, 4, DE], f32r)
            nc.vector.tensor_copy(sb_w1t, sb_w1t_f)
            sb_w23t_f = consts.tile([128, 4, 2 * DE], f32)
            nc.sync.dma_start(out=sb_w23t_f, in_=w23t.ap().rearrange("(kt p) d -> p kt d", p=128))
            sb_w23t = consts.tile([128, 4, 2 * DE], f32r)
            nc.vector.tensor_copy(sb_w23t, sb_w23t_f)
            sb_wot_f = consts.tile([DE, DIN], f32)
            nc.sync.dma_start(out=sb_wot_f, in_=wot.ap())
            sb_wot = consts.tile([DE, DIN], f32r)
            nc.vector.tensor_copy(sb_wot, sb_wot_f)
            sb_b1 = consts.tile([DE, 1], f32)
            nc.sync.dma_start(out=sb_b1, in_=b1.ap())
            sb_b23 = consts.tile([2 * DE, 1], f32)
            nc.sync.dma_start(out=sb_b23, in_=b23.ap())
            sb_bo = consts.tile([1, DIN], f32)
            nc.sync.dma_start(out=sb_bo, in_=bo.ap())
            identity = consts.tile([128, 128], f32)
            nc.sync.dma_start(out=identity, in_=ident.ap())
            # per-core group-select mask, broadcast to [DE, n_cores]
            maskt = consts.tile([DE, n_cores], f32)
            import concourse.bass as bass_mod

            kvm_ap = kvmask.ap()
            kvm_bcast = bass_mod.AP(
                tensor=kvm_ap.tensor, offset=kvm_ap.offset,
                ap=[[0, DE], kvm_ap.ap[-1]],
            )
            nc.gpsimd.dma_start(out=maskt, in_=kvm_bcast)

            # e1^T for all local rows, with a row of ones at partition DE so the
            # final matmul folds in the output bias (lhsT K = DE+1).
            e1t = persist.tile([128, rows], f32r)
            ones_row = consts.tile([1, rows], f32)
            nc.vector.memset(ones_row, 1.0)
            nc.vector.tensor_copy(e1t[DE : DE + 1, :], ones_row)
            # M = KV @ WO.T in rows 0..63, bO in row DE.
            mmat = persist.tile([128, DIN], f32r)
            nc.vector.tensor_copy(mmat[DE : DE + 1, :], sb_bo)

            kvt_ps = kvps.tile([DE, DE], f32)  # accumulates e3^T @ e2 over all tiles

            # ---- Phase A: load x^T, cast to f32r, project, partial KV^T ----
            with (
                tc.tile_pool(name="xf", bufs=3) as xfp,
                tc.tile_pool(name="xr", bufs=3) as xrp,
                tc.tile_pool(name="e23tps", bufs=2, space="PSUM") as e23tpsp,
                tc.tile_pool(name="e23tsb", bufs=2) as e23tsbp,
                tc.tile_pool(name="trps", bufs=2, space="PSUM") as trpsp,
                tc.tile_pool(name="e23n", bufs=2) as e23np,
                tc.tile_pool(name="e1ps", bufs=2, space="PSUM") as e1psp,
            ):
                for j in range(n_chunks):
                    xf = xfp.tile([128, 4, 512], f32)
                    nc.sync.dma_start(out=xf, in_=xt_t[:, :, j * 512 : (j + 1) * 512])
                    xr = xrp.tile([128, 4, 512], f32r)
                    nc.vector.tensor_copy(xr, xf)

                    # e23T = [W2;W3] @ x^T  -> [128, 512] (d on partitions)
                    e23t_ps = e23tpsp.tile([128, 512], f32)
                    for kt in range(4):
                        nc.tensor.matmul(
                            e23t_ps,
                            lhsT=sb_w23t[:, kt, :],
                            rhs=xr[:, kt, :],
                            start=(kt == 0),
                            stop=(kt == 3),
                        )
                    e23t_sb = e23tsbp.tile([128, 512], f32)
                    nc.scalar.activation(
                        e23t_sb,
                        e23t_ps,
                        mybir.ActivationFunctionType.Identity,
                        bias=sb_b23,
                        scale=1.0,
                    )

                    # e1T = W1 @ x^T -> [64, 512], + b1, kept f32r for phase C
                    e1_ps = e1psp.tile([DE, 512], f32)
                    for kt in range(4):
                        nc.tensor.matmul(
                            e1_ps,
                            lhsT=sb_w1t[:, kt, :],
                            rhs=xr[:, kt, :],
                            start=(kt == 0),
                            stop=(kt == 3),
                        )
                    nc.vector.tensor_scalar_add(
                        e1t[:DE, j * 512 : (j + 1) * 512], e1_ps, sb_b1
                    )

                    # transpose e23T back to natural layout (full f32, batched
                    # into one PSUM bank -> single DVE copy), accumulate
                    # KV^T = e3^T @ e2
                    tr_ps = trpsp.tile([128, 512], f32)
                    for t in range(4):
                        nc.tensor.transpose(
                            tr_ps[:, t * 128 : (t + 1) * 128],
                            e23t_sb[:, t * 128 : (t + 1) * 128],
                            identity[:, :],
                        )
                    e23n = e23np.tile([128, 512], f32)
                    nc.vector.tensor_copy(e23n, tr_ps)
                    for t in range(4):
                        tt = j * 4 + t
                        nc.tensor.matmul(
                            kvt_ps,
                            lhsT=e23n[:, t * 128 + DE : (t + 1) * 128],
                            rhs=e23n[:, t * 128 : t * 128 + DE],
                            start=(tt == 0),
                            stop=(tt == 4 * n_chunks - 1),
                        )

            # ---- Collective: AllGather partial KV^T across all cores ----
            with (
                tc.tile_pool(name="small", bufs=1) as small,
                tc.tile_pool(name="mmps", bufs=1, space="PSUM") as mmpsp,
            ):
                kvt_sb = small.tile([DE, DE], f32)
                nc.vector.tensor_copy(kvt_sb, kvt_ps)
                cc_in = dram.tile([DE, DE], f32)
                cc_out = dram.tile([n_cores, DE, DE], f32)
                nc.gpsimd.dma_start(out=cc_in[:, :], in_=kvt_sb)
                nc.gpsimd.collective_compute(
                    "AllGather",
                    mybir.AluOpType.bypass,
                    replica_groups=[list(range(n_cores))],
                    ins=[cc_in[:, :]],
                    outs=[cc_out[:, :, :]],
                )
                kvt_all = small.tile([DE, n_cores, DE], f32)
                nc.sync.dma_start(
                    out=kvt_all, in_=cc_out[:, :, :].rearrange("r p d -> p r d")
                )
                # mask-weighted sum of the n_cores partials (mask selects the
                # four cores of this batch); final op rounds to f32r
                kvt_r = small.tile([DE, DE], f32r)
                acc = small.tile([DE, DE], f32)
                nc.vector.tensor_scalar_mul(acc, kvt_all[:, 0, :], maskt[:, 0:1])
                for r in range(1, n_cores - 1):
                    nc.vector.scalar_tensor_tensor(
                        acc, kvt_all[:, r, :], maskt[:, r : r + 1], acc,
                        op0=mult, op1=add,
                    )
                nc.vector.scalar_tensor_tensor(
                    kvt_r, kvt_all[:, n_cores - 1, :],
                    maskt[:, n_cores - 1 : n_cores], acc,
                    op0=mult, op1=add,
                )
                mm_ps = mmpsp.tile([DE, DIN], f32)
                nc.tensor.matmul(mm_ps, lhsT=kvt_r, rhs=sb_wot)
                nc.vector.tensor_copy(mmat[:DE, :], mm_ps)

            # ---- Phase C: out = sigmoid(e1 @ M + bO) ----
            with (
                tc.tile_pool(name="ops", bufs=2, space="PSUM") as opsp,
                tc.tile_pool(name="osb", bufs=2) as osbp,
            ):
                for j in range(n_chunks):
                    osb = osbp.tile([128, 4, DIN], f32)
                    for t in range(4):
                        tt = j * 4 + t
                        o_ps = opsp.tile([128, DIN], f32)
                        nc.tensor.matmul(
                            o_ps,
                            lhsT=e1t[: DE + 1, tt * 128 : (tt + 1) * 128],
                            rhs=mmat[: DE + 1, :],
                        )
                        nc.scalar.activation(
                            osb[:, t, :], o_ps, mybir.ActivationFunctionType.Sigmoid
                        )
                    nc.sync.dma_start(
                        out=out.ap()[j * 512 : (j + 1) * 512, :].rearrange(
                            "(t p) o -> p t o", p=128
                        ),
                        in_=osb,
                    )
    nc.compile()
    return nc


def make_in_maps(x, W1, b1, W2, b2, W3, b3, WO, bO, rows=ROWS, n_cores=N_CORES):
    x = np.asarray(x, dtype=np.float32)
    total = x.shape[0] * x.shape[1]
    xt_full = np.ascontiguousarray(x.reshape(total, DIN).T)  # [512, total]
    shared = {
        "w1t": np.ascontiguousarray(np.asarray(W1, np.float32).T),
        "w23t": np.ascontiguousarray(
            np.concatenate(
                [np.asarray(W2, np.float32).T, np.asarray(W3, np.float32).T], axis=1
            )
        ),
        "wot": np.ascontiguousarray(np.asarray(WO, np.float32).T),
        "b1": np.ascontiguousarray(np.asarray(b1, np.float32).reshape(DE, 1)),
        "b23": np.ascontiguousarray(
            np.concatenate([np.asarray(b2, np.float32), np.asarray(b3, np.float32)])
        ).reshape(2 * DE, 1),
        "bo": np.ascontiguousarray(np.asarray(bO, np.float32).reshape(1, DIN)),
        "ident": np.eye(128, dtype=np.float32),
    }
    half = n_cores // 2
    in_maps = []
    for c in range(n_cores):
        m = dict(shared)
        m["xt"] = np.ascontiguousarray(xt_full[:, c * rows : (c + 1) * rows])
        mask = np.zeros((1, n_cores), np.float32)
        if c < half:
            mask[0, :half] = 1.0
        else:
            mask[0, half:] = 1.0
        m["kvmask"] = mask
        in_maps.append(m)
    return in_maps


def kernel(x, W1, b1, W2, b2, W3, b3, WO, bO):
    global LAST_RESULT
    if "nc" not in _NC_CACHE:
        _NC_CACHE["nc"] = build_nc()
    nc = _NC_CACHE["nc"]
    in_maps = make_in_maps(x, W1, b1, W2, b2, W3, b3, WO, bO)
    res = run_bass_kernel_spmd(
        nc,
        in_maps,
        core_ids=list(range(N_CORES)),
        trace=TRACE,
        **TRACE_KWARGS,
    )
    LAST_RESULT = res
    full = np.concatenate(
        [res.results[c]["out"] for c in range(N_CORES)], axis=0
    )  # [16384, 512] f32
    return full.reshape(BATCH, SEQ, DIN)


# revision 10
# speedup vs baseline: 1.0585x; 1.0585x over previous
# Trainium2 Bass kernel for nn_Attention3 (unnormalized linear attention).
#
# Math: e_i = x @ W_i.T + b_i (i=1,2,3);  out = sigmoid((e1 @ e2.T @ e3) @ WO.T + bO)
# Since there is no softmax, (e1 @ e2.T) @ e3 == e1 @ (e2.T @ e3) where
# KV = e2.T @ e3 is only [64, 64].  The kernel is therefore memory-bound:
# read x once, write out once.
#
# Sharding: the flattened [B*S, 512] = [16384, 512] rows are split into 8
# contiguous chunks of 2048 rows (cores 0-3 <- batch 0, cores 4-7 <- batch 1).
# Each core computes its partial KV^T = e3_c.T @ e2_c over its rows, the four
# cores of a batch AllGather+sum their partials, then each core finishes
# out = sigmoid(e1 @ (KV @ WO.T) + bO) for its rows.  The e1 projection is
# scheduled after the KV^T chain so the PE computes it during the collective.
#
# Precision/layout: x arrives host-transposed ([512, rows] f32) so the
# contraction dim sits on SBUF partitions, and is cast on-chip to float32r
# (tf32-grade, full PE rate at free-dim >= 256).  Projections are computed
# transposed ([64|128, 512] out tiles, N=512) in f32r; e2|e3 is bias-added and
# PE-transposed back to natural layout in full f32 for the KV^T accumulation.
# The final e1 @ (KV @ WO.T) matmul runs in f32r at N=512.
#
# All constants arrive packed in one [128, 1922] f32 blob (single DMA).

import numpy as np

import concourse.mybir as mybir
import concourse.tile as tile
from concourse import bacc
from concourse.bass_utils import run_bass_kernel_spmd

BATCH = 2
SEQ = 8192
DIN = 512
DE = 64
N_CORES = 8
ROWS = (BATCH * SEQ) // N_CORES  # 2048 rows per core

# const blob layout (free-dim offsets, f32, [128, NB])
_OFF_W1T = 0          # [128, 4, 64]   w1t rearranged (kt p) d -> p kt d
_OFF_W23T = 256       # [128, 4, 128]  w23t rearranged
_OFF_IDENT = 768      # [128, 128]     identity
_OFF_B23 = 896        # [128, 1]       b2|b3 (per-partition)
_OFF_WOT = 897        # [64, 512]      WO.T (rows 0..63)
_OFF_B1 = 1409        # [64, 1]        b1 (rows 0..63)
_OFF_BO = 1410        # [1, 512]       bO (row 0)
_NB = 1922

TRACE = False
TRACE_KWARGS = {}
LAST_RESULT = None

_NC_CACHE = {}


def build_nc(rows=ROWS, n_cores=N_CORES):
    f32 = mybir.dt.float32
    f32r = mybir.dt.float32r

    half = n_cores // 2
    groups = [list(range(half)), list(range(half, n_cores))]
    ngrp = half

    assert rows % 512 == 0
    n_chunks = rows // 512

    nc = bacc.Bacc(None, target_bir_lowering=False, debug=False, num_devices=n_cores)

    xt = nc.dram_tensor("xt", [DIN, rows], f32, kind="ExternalInput")
    wconst = nc.dram_tensor("wconst", [128, _NB], f32, kind="ExternalInput")
    out = nc.dram_tensor("out", [rows, DIN], f32, kind="ExternalOutput")

    xt_t = xt.ap().rearrange("(kt p) s -> p kt s", p=128)  # [128, 4, rows]

    with tile.TileContext(nc) as tc:
        with (
            tc.tile_pool(name="consts", bufs=1) as consts,
            tc.tile_pool(name="persist", bufs=1) as persist,
            tc.tile_pool(name="kvps", bufs=1, space="PSUM") as kvps,
            tc.tile_pool(name="dram", bufs=1, space="DRAM") as dram,
        ):
            blob = consts.tile([128, _NB], f32)
            nc.sync.dma_start(out=blob, in_=wconst.ap())

            sb_w1t = consts.tile([128, 4, DE], f32r)
            nc.vector.tensor_copy(
                sb_w1t, blob[:, _OFF_W1T : _OFF_W1T + 256].rearrange("p (kt d) -> p kt d", kt=4)
            )
            sb_w23t = consts.tile([128, 4, 2 * DE], f32r)
            nc.vector.tensor_copy(
                sb_w23t,
                blob[:, _OFF_W23T : _OFF_W23T + 512].rearrange("p (kt d) -> p kt d", kt=4),
            )
            sb_wot = consts.tile([DE, DIN], f32r)
            nc.vector.tensor_copy(sb_wot, blob[:DE, _OFF_WOT : _OFF_WOT + DIN])
            identity = blob[:, _OFF_IDENT : _OFF_IDENT + 128]
            sb_b23 = blob[:, _OFF_B23 : _OFF_B23 + 1]
            sb_b1 = blob[:DE, _OFF_B1 : _OFF_B1 + 1]
            sb_bo = blob[:1, _OFF_BO : _OFF_BO + DIN]

            # e1^T for all local rows, with a row of ones at partition DE so the
            # final matmul folds in the output bias (lhsT K = DE+1).
            e1t = persist.tile([128, rows], f32r)
            ones_row = consts.tile([1, rows], f32)
            nc.vector.memset(ones_row, 1.0)
            nc.vector.tensor_copy(e1t[DE : DE + 1, :], ones_row)
            # M = KV @ WO.T in rows 0..63, bO in row DE.
            mmat = persist.tile([128, DIN], f32r)
            nc.vector.tensor_copy(mmat[DE : DE + 1, :], sb_bo)

            kvt_ps = kvps.tile([DE, DE], f32)  # accumulates e3^T @ e2 over all tiles

            # ---- Phase A: load x^T, cast to f32r, e2|e3 -> partial KV^T ----
            with (
                tc.tile_pool(name="xf", bufs=2) as xfp,
                tc.tile_pool(name="xr", bufs=4) as xrp,
                tc.tile_pool(name="e23tps", bufs=2, space="PSUM") as e23tpsp,
                tc.tile_pool(name="e23tsb", bufs=2) as e23tsbp,
                tc.tile_pool(name="trps", bufs=2, space="PSUM") as trpsp,
                tc.tile_pool(name="e23n", bufs=2) as e23np,
                tc.tile_pool(name="e1ps", bufs=2, space="PSUM") as e1psp,
                tc.tile_pool(name="small", bufs=1) as small,
                tc.tile_pool(name="mmps", bufs=1, space="PSUM") as mmpsp,
            ):
                xrs = []
                for j in range(n_chunks):
                    xf = xfp.tile([128, 4, 512], f32)
                    nc.sync.dma_start(out=xf, in_=xt_t[:, :, j * 512 : (j + 1) * 512])
                    xr = xrp.tile([128, 4, 512], f32r)
                    nc.vector.tensor_copy(xr, xf)
                    xrs.append(xr)

                    # e23T = [W2;W3] @ x^T  -> [128, 512] (d on partitions)
                    e23t_ps = e23tpsp.tile([128, 512], f32)
                    for kt in range(4):
                        nc.tensor.matmul(
                            e23t_ps,
                            lhsT=sb_w23t[:, kt, :],
                            rhs=xr[:, kt, :],
                            start=(kt == 0),
                            stop=(kt == 3),
                        )
                    e23t_sb = e23tsbp.tile([128, 512], f32)
                    nc.scalar.activation(
                        e23t_sb,
                        e23t_ps,
                        mybir.ActivationFunctionType.Identity,
                        bias=sb_b23,
                        scale=1.0,
                    )

                    # transpose e23T back to natural layout (full f32, batched
                    # into one PSUM bank -> single DVE copy), accumulate
                    # KV^T = e3^T @ e2
                    tr_ps = trpsp.tile([128, 512], f32)
                    for t in range(4):
                        nc.tensor.transpose(
                            tr_ps[:, t * 128 : (t + 1) * 128],
                            e23t_sb[:, t * 128 : (t + 1) * 128],
                            identity,
                        )
                    e23n = e23np.tile([128, 512], f32)
                    nc.vector.tensor_copy(e23n, tr_ps)
                    for t in range(4):
                        tt = j * 4 + t
                        nc.tensor.matmul(
                            kvt_ps,
                            lhsT=e23n[:, t * 128 + DE : (t + 1) * 128],
                            rhs=e23n[:, t * 128 : t * 128 + DE],
                            start=(tt == 0),
                            stop=(tt == 4 * n_chunks - 1),
                        )

                # ---- Collective (HWDGE path): AllGather partial KV^T ----
                kvt_sb = small.tile([DE, DE], f32)
                nc.vector.tensor_copy(kvt_sb, kvt_ps)
                cc_in = dram.tile([DE, DE], f32)
                cc_out = dram.tile([ngrp, DE, DE], f32)
                nc.sync.dma_start(out=cc_in[:, :], in_=kvt_sb)
                nc.gpsimd.collective_compute(
                    "AllGather",
                    mybir.AluOpType.bypass,
                    replica_groups=groups,
                    ins=[cc_in[:, :]],
                    outs=[cc_out[:, :, :]],
                )

                # ---- e1T = W1 @ x^T (+b1): runs on PE during the collective ----
                for j in range(n_chunks):
                    e1_ps = e1psp.tile([DE, 512], f32)
                    for kt in range(4):
                        nc.tensor.matmul(
                            e1_ps,
                            lhsT=sb_w1t[:, kt, :],
                            rhs=xrs[j][:, kt, :],
                            start=(kt == 0),
                            stop=(kt == 3),
                        )
                    nc.vector.tensor_scalar_add(
                        e1t[:DE, j * 512 : (j + 1) * 512], e1_ps, sb_b1
                    )

                # ---- collect AllGather result, M = KV @ WO.T ----
                kvt_all = small.tile([DE, ngrp, DE], f32)
                nc.sync.dma_start(
                    out=kvt_all, in_=cc_out[:, :, :].rearrange("r p d -> p r d")
                )
                kvt_r = small.tile([DE, DE], f32r)
                if ngrp == 4:
                    s01 = small.tile([DE, DE], f32)
                    nc.vector.tensor_add(s01, kvt_all[:, 0, :], kvt_all[:, 1, :])
                    s23 = small.tile([DE, DE], f32)
                    nc.vector.tensor_add(s23, kvt_all[:, 2, :], kvt_all[:, 3, :])
                    nc.vector.tensor_add(kvt_r, s01, s23)
                elif ngrp == 2:
                    nc.vector.tensor_add(kvt_r, kvt_all[:, 0, :], kvt_all[:, 1, :])
                else:
                    nc.vector.tensor_copy(kvt_r, kvt_all[:, 0, :])
                mm_ps = mmpsp.tile([DE, DIN], f32)
                nc.tensor.matmul(mm_ps, lhsT=kvt_r, rhs=sb_wot)
                nc.vector.tensor_copy(mmat[:DE, :], mm_ps)

            # ---- Phase C: out = sigmoid(e1 @ M + bO) ----
            with (
                tc.tile_pool(name="ops", bufs=2, space="PSUM") as opsp,
                tc.tile_pool(name="osb", bufs=2) as osbp,
            ):
                for j in range(n_chunks):
                    osb = osbp.tile([128, 4, DIN], f32)
                    for t in range(4):
                        tt = j * 4 + t
                        o_ps = opsp.tile([128, DIN], f32)
                        nc.tensor.matmul(
                            o_ps,
                            lhsT=e1t[: DE + 1, tt * 128 : (tt + 1) * 128],
                            rhs=mmat[: DE + 1, :],
                        )
                        nc.scalar.activation(
                            osb[:, t, :], o_ps, mybir.ActivationFunctionType.Sigmoid
                        )
                    nc.sync.dma_start(
                        out=out.ap()[j * 512 : (j + 1) * 512, :].rearrange(
                            "(t p) o -> p t o", p=128
                        ),
                        in_=osb,
                    )
    nc.compile()
    return nc


def make_wconst(W1, b1, W2, b2, W3, b3, WO, bO):
    blob = np.zeros((128, _NB), np.float32)
    w1t = np.asarray(W1, np.float32).T.reshape(4, 128, DE)  # (kt, p, d)
    blob[:, _OFF_W1T : _OFF_W1T + 256] = (
        w1t.transpose(1, 0, 2).reshape(128, 4 * DE)
    )
    w23t = np.concatenate(
        [np.asarray(W2, np.float32).T, np.asarray(W3, np.float32).T], axis=1
    ).reshape(4, 128, 2 * DE)
    blob[:, _OFF_W23T : _OFF_W23T + 512] = (
        w23t.transpose(1, 0, 2).reshape(128, 8 * DE)
    )
    blob[:, _OFF_IDENT : _OFF_IDENT + 128] = np.eye(128, dtype=np.float32)
    blob[:, _OFF_B23] = np.concatenate(
        [np.asarray(b2, np.float32), np.asarray(b3, np.float32)]
    )
    blob[:DE, _OFF_WOT : _OFF_WOT + DIN] = np.asarray(WO, np.float32).T
    blob[:DE, _OFF_B1] = np.asarray(b1, np.float32)
    blob[0, _OFF_BO : _OFF_BO + DIN] = np.asarray(bO, np.float32)
    return blob


def make_in_maps(x, W1, b1, W2, b2, W3, b3, WO, bO, rows=ROWS, n_cores=N_CORES):
    x = np.asarray(x, dtype=np.float32)
    total = x.shape[0] * x.shape[1]
    xt_full = np.ascontiguousarray(x.reshape(total, DIN).T)  # [512, total]
    blob = make_wconst(W1, b1, W2, b2, W3, b3, WO, bO)
    in_maps = []
    for c in range(n_cores):
        in_maps.append(
            {
                "wconst": blob,
                "xt": np.ascontiguousarray(xt_full[:, c * rows : (c + 1) * rows]),
            }
        )
    return in_maps


def kernel(x, W1, b1, W2, b2, W3, b3, WO, bO):
    global LAST_RESULT
    if "nc" not in _NC_CACHE:
        _NC_CACHE["nc"] = build_nc()
    nc = _NC_CACHE["nc"]
    in_maps = make_in_maps(x, W1, b1, W2, b2, W3, b3, WO, bO)
    res = run_bass_kernel_spmd(
        nc,
        in_maps,
        core_ids=list(range(N_CORES)),
        trace=TRACE,
        **TRACE_KWARGS,
    )
    LAST_RESULT = res
    full = np.concatenate(
        [res.results[c]["out"] for c in range(N_CORES)], axis=0
    )  # [16384, 512] f32
    return full.reshape(BATCH, SEQ, DIN)


# revision 14
# speedup vs baseline: 1.1815x; 1.1163x over previous
# Trainium2 Bass kernel for nn_Attention3 (unnormalized linear attention).
#
# Math: e_i = x @ W_i.T + b_i (i=1,2,3);  out = sigmoid((e1 @ e2.T @ e3) @ WO.T + bO)
# Since there is no softmax, (e1 @ e2.T) @ e3 == e1 @ (e2.T @ e3) where
# KV = e2.T @ e3 is only [64, 64].  The kernel is therefore memory-bound:
# read x once, write out once.
#
# Sharding: the flattened [B*S, 512] = [16384, 512] rows are split into 8
# contiguous chunks of 2048 rows (cores 0-3 <- batch 0, cores 4-7 <- batch 1).
# Each core computes its partial KV^T = e3_c.T @ e2_c over its rows, the four
# cores of a batch AllGather+sum their partials, then each core finishes
# out = sigmoid(e1 @ (KV @ WO.T) + bO) for its rows.  The e1 projection is
# scheduled after the KV^T chain so the PE computes it during the collective.
#
# Precision/layout: x arrives host-transposed ([512, rows] f32) so the
# contraction dim sits on SBUF partitions, and is cast on-chip to float32r
# (tf32-grade, full PE rate at free-dim >= 256).  Projections are computed
# transposed ([64|128, 512] out tiles, N=512) in f32r; e2|e3 is bias-added and
# PE-transposed back to natural layout in full f32 for the KV^T accumulation.
# The final e1 @ (KV @ WO.T) matmul runs in f32r at N=512.
#
# All constants arrive packed in one [128, 1922] f32 blob (single DMA).

import numpy as np

import concourse.mybir as mybir
import concourse.tile as tile
from concourse import bacc
from concourse.bass_utils import run_bass_kernel_spmd

BATCH = 2
SEQ = 8192
DIN = 512
DE = 64
N_CORES = 8
ROWS = (BATCH * SEQ) // N_CORES  # 2048 rows per core

# const blob layout (free-dim offsets, f32, [128, NB])
_OFF_W1T = 0          # [128, 4, 64]   w1t rearranged (kt p) d -> p kt d
_OFF_W23T = 256       # [128, 4, 128]  w23t rearranged
_OFF_IDENT = 768      # [128, 128]     identity
_OFF_B23 = 896        # [128, 1]       b2|b3 (per-partition)
_OFF_WOT = 897        # [64, 512]      WO.T (rows 0..63)
_OFF_B1 = 1409        # [64, 1]        b1 (rows 0..63)
_OFF_BO = 1410        # [1, 512]       bO (row 0)
_NB = 1922

TRACE = False
TRACE_KWARGS = {}
LAST_RESULT = None

_NC_CACHE = {}


def build_nc(rows=ROWS, n_cores=N_CORES):
    f32 = mybir.dt.float32
    f32r = mybir.dt.float32r

    half = n_cores // 2
    groups = [list(range(half)), list(range(half, n_cores))]
    ngrp = half

    assert rows % 512 == 0
    n_chunks = rows // 512

    nc = bacc.Bacc(
        None,
        target_bir_lowering=False,
        debug=False,
        num_devices=n_cores,
        enable_partition_id=False,
    )

    xt = nc.dram_tensor("xt", [DIN, rows], f32, kind="ExternalInput")
    wconst = nc.dram_tensor("wconst", [128, _NB], f32, kind="ExternalInput")
    out = nc.dram_tensor("out", [rows, DIN], f32, kind="ExternalOutput")

    xt_t = xt.ap().rearrange("(kt p) s -> p kt s", p=128)  # [128, 4, rows]

    with tile.TileContext(nc) as tc:
        with (
            tc.tile_pool(name="consts", bufs=1) as consts,
            tc.tile_pool(name="persist", bufs=1) as persist,
            tc.tile_pool(name="kvps", bufs=1, space="PSUM") as kvps,
            tc.tile_pool(name="dram", bufs=1, space="DRAM") as dram,
            tc.tile_pool(name="small", bufs=1) as small,
        ):
            blob = consts.tile([128, _NB], f32)
            nc.sync.dma_start(out=blob, in_=wconst.ap())

            sb_w1t = consts.tile([128, 4, DE], f32r)
            nc.vector.tensor_copy(
                sb_w1t, blob[:, _OFF_W1T : _OFF_W1T + 256].rearrange("p (kt d) -> p kt d", kt=4)
            )
            sb_w23t = consts.tile([128, 4, 2 * DE], f32r)
            nc.vector.tensor_copy(
                sb_w23t,
                blob[:, _OFF_W23T : _OFF_W23T + 512].rearrange("p (kt d) -> p kt d", kt=4),
            )
            sb_wot = consts.tile([DE, DIN], f32r)
            nc.vector.tensor_copy(sb_wot, blob[:DE, _OFF_WOT : _OFF_WOT + DIN])
            identity = blob[:, _OFF_IDENT : _OFF_IDENT + 128]
            sb_b23 = blob[:, _OFF_B23 : _OFF_B23 + 1]
            sb_b1 = blob[:DE, _OFF_B1 : _OFF_B1 + 1]
            sb_bo = blob[:1, _OFF_BO : _OFF_BO + DIN]

            # e1^T for all local rows, with a row of ones at partition DE so the
            # final matmul folds in the output bias (lhsT K = DE+1).
            e1t = persist.tile([128, rows], f32r)
            ones_row = consts.tile([1, rows], f32)
            nc.vector.memset(ones_row, 1.0)
            nc.vector.tensor_copy(e1t[DE : DE + 1, :], ones_row)
            # M = KV @ WO.T in rows 0..63, bO in row DE.
            mmat = persist.tile([128, DIN], f32r)
            nc.vector.tensor_copy(mmat[DE : DE + 1, :], sb_bo)

            kvt_ps = kvps.tile([DE, DE], f32)  # accumulates e3^T @ e2 over all tiles

            # ---- Phase A: load x^T, cast to f32r, e2|e3 -> partial KV^T ----
            with (
                tc.tile_pool(name="xf", bufs=2) as xfp,
                tc.tile_pool(name="xr", bufs=4) as xrp,
                tc.tile_pool(name="e23tps", bufs=3, space="PSUM") as e23tpsp,
                tc.tile_pool(name="e23tsb", bufs=2) as e23tsbp,
                tc.tile_pool(name="trps", bufs=2, space="PSUM") as trpsp,
                tc.tile_pool(name="e23n", bufs=2) as e23np,
                tc.tile_pool(name="e1ps", bufs=2, space="PSUM") as e1psp,
            ):
                xrs = []
                for j in range(n_chunks):
                    xf = xfp.tile([128, 4, 512], f32)
                    nc.sync.dma_start(out=xf, in_=xt_t[:, :, j * 512 : (j + 1) * 512])
                    xr = xrp.tile([128, 4, 512], f32r)
                    nc.vector.tensor_copy(xr, xf)
                    xrs.append(xr)

                    # e23T = [W2;W3] @ x^T  -> [128, 512] (d on partitions)
                    e23t_ps = e23tpsp.tile([128, 512], f32)
                    for kt in range(4):
                        nc.tensor.matmul(
                            e23t_ps,
                            lhsT=sb_w23t[:, kt, :],
                            rhs=xr[:, kt, :],
                            start=(kt == 0),
                            stop=(kt == 3),
                        )
                    e23t_sb = e23tsbp.tile([128, 512], f32)
                    nc.scalar.activation(
                        e23t_sb,
                        e23t_ps,
                        mybir.ActivationFunctionType.Identity,
                        bias=sb_b23,
                        scale=1.0,
                    )

                    # transpose e23T back to natural layout (full f32, batched
                    # into one PSUM bank -> single DVE copy), accumulate
                    # KV^T = e3^T @ e2
                    tr_ps = trpsp.tile([128, 512], f32)
                    for t in range(4):
                        nc.tensor.transpose(
                            tr_ps[:, t * 128 : (t + 1) * 128],
                            e23t_sb[:, t * 128 : (t + 1) * 128],
                            identity,
                        )
                    e23n = e23np.tile([128, 512], f32)
                    nc.vector.tensor_copy(e23n, tr_ps)
                    for t in range(4):
                        tt = j * 4 + t
                        nc.tensor.matmul(
                            kvt_ps,
                            lhsT=e23n[:, t * 128 + DE : (t + 1) * 128],
                            rhs=e23n[:, t * 128 : t * 128 + DE],
                            start=(tt == 0),
                            stop=(tt == 4 * n_chunks - 1),
                        )

                # ---- Collective (HWDGE path): AllGather partial KV^T ----
                kvt_sb = small.tile([DE, DE], f32)
                nc.vector.tensor_copy(kvt_sb, kvt_ps)
                cc_in = dram.tile([DE, DE], f32)
                cc_out = dram.tile([ngrp, DE, DE], f32)
                nc.sync.dma_start(out=cc_in[:, :], in_=kvt_sb)
                nc.gpsimd.collective_compute(
                    "AllGather",
                    mybir.AluOpType.bypass,
                    replica_groups=groups,
                    ins=[cc_in[:, :]],
                    outs=[cc_out[:, :, :]],
                )

                # ---- e1T = W1 @ x^T (+b1): runs on PE during the collective ----
                for j in range(n_chunks):
                    e1_ps = e1psp.tile([DE, 512], f32)
                    for kt in range(4):
                        nc.tensor.matmul(
                            e1_ps,
                            lhsT=sb_w1t[:, kt, :],
                            rhs=xrs[j][:, kt, :],
                            start=(kt == 0),
                            stop=(kt == 3),
                        )
                    nc.vector.tensor_scalar_add(
                        e1t[:DE, j * 512 : (j + 1) * 512], e1_ps, sb_b1
                    )

            # ---- collect AllGather result, M = KV @ WO.T ----
            with tc.tile_pool(name="mmps", bufs=1, space="PSUM") as mmpsp:
                kvt_all = small.tile([DE, ngrp, DE], f32)
                nc.sync.dma_start(
                    out=kvt_all, in_=cc_out[:, :, :].rearrange("r p d -> p r d")
                )
                kvt_r = small.tile([DE, DE], f32r)
                if ngrp == 4:
                    s01 = small.tile([DE, DE], f32)
                    nc.vector.tensor_add(s01, kvt_all[:, 0, :], kvt_all[:, 1, :])
                    s23 = small.tile([DE, DE], f32)
                    nc.vector.tensor_add(s23, kvt_all[:, 2, :], kvt_all[:, 3, :])
                    nc.vector.tensor_add(kvt_r, s01, s23)
                elif ngrp == 2:
                    nc.vector.tensor_add(kvt_r, kvt_all[:, 0, :], kvt_all[:, 1, :])
                else:
                    nc.vector.tensor_copy(kvt_r, kvt_all[:, 0, :])
                mm_ps = mmpsp.tile([DE, DIN], f32)
                nc.tensor.matmul(mm_ps, lhsT=kvt_r, rhs=sb_wot)
                nc.vector.tensor_copy(mmat[:DE, :], mm_ps)

            # ---- Phase C: out = sigmoid(e1 @ M + bO) ----
            with (
                tc.tile_pool(name="ops", bufs=2, space="PSUM") as opsp,
                tc.tile_pool(name="osb", bufs=2) as osbp,
            ):
                for j in range(n_chunks):
                    osb = osbp.tile([128, 4, DIN], f32)
                    for t in range(4):
                        tt = j * 4 + t
                        o_ps = opsp.tile([128, DIN], f32)
                        nc.tensor.matmul(
                            o_ps,
                            lhsT=e1t[: DE + 1, tt * 128 : (tt + 1) * 128],
                            rhs=mmat[: DE + 1, :],
                        )
                        nc.scalar.activation(
                            osb[:, t, :], o_ps, mybir.ActivationFunctionType.Sigmoid
                        )
                    nc.sync.dma_start(
                        out=out.ap()[j * 512 : (j + 1) * 512, :].rearrange(
                            "(t p) o -> p t o", p=128
                        ),
                        in_=osb,
                    )
    nc.compile()
    return nc


def make_wconst(W1, b1, W2, b2, W3, b3, WO, bO):
    blob = np.zeros((128, _NB), np.float32)
    w1t = np.asarray(W1, np.float32).T.reshape(4, 128, DE)  # (kt, p, d)
    blob[:, _OFF_W1T : _OFF_W1T + 256] = (
        w1t.transpose(1, 0, 2).reshape(128, 4 * DE)
    )
    w23t = np.concatenate(
        [np.asarray(W2, np.float32).T, np.asarray(W3, np.float32).T], axis=1
    ).reshape(4, 128, 2 * DE)
    blob[:, _OFF_W23T : _OFF_W23T + 512] = (
        w23t.transpose(1, 0, 2).reshape(128, 8 * DE)
    )
    blob[:, _OFF_IDENT : _OFF_IDENT + 128] = np.eye(128, dtype=np.float32)
    blob[:, _OFF_B23] = np.concatenate(
        [np.asarray(b2, np.float32), np.asarray(b3, np.float32)]
    )
    blob[:DE, _OFF_WOT : _OFF_WOT + DIN] = np.asarray(WO, np.float32).T
    blob[:DE, _OFF_B1] = np.asarray(b1, np.float32)
    blob[0, _OFF_BO : _OFF_BO + DIN] = np.asarray(bO, np.float32)
    return blob


def make_in_maps(x, W1, b1, W2, b2, W3, b3, WO, bO, rows=ROWS, n_cores=N_CORES):
    x = np.asarray(x, dtype=np.float32)
    total = x.shape[0] * x.shape[1]
    xt_full = np.ascontiguousarray(x.reshape(total, DIN).T)  # [512, total]
    blob = make_wconst(W1, b1, W2, b2, W3, b3, WO, bO)
    in_maps = []
    for c in range(n_cores):
        in_maps.append(
            {
                "wconst": blob,
                "xt": np.ascontiguousarray(xt_full[:, c * rows : (c + 1) * rows]),
            }
        )
    return in_maps


def kernel(x, W1, b1, W2, b2, W3, b3, WO, bO):
    global LAST_RESULT
    if "nc" not in _NC_CACHE:
        _NC_CACHE["nc"] = build_nc()
    nc = _NC_CACHE["nc"]
    in_maps = make_in_maps(x, W1, b1, W2, b2, W3, b3, WO, bO)
    res = run_bass_kernel_spmd(
        nc,
        in_maps,
        core_ids=list(range(N_CORES)),
        trace=TRACE,
        **TRACE_KWARGS,
    )
    LAST_RESULT = res
    full = np.concatenate(
        [res.results[c]["out"] for c in range(N_CORES)], axis=0
    )  # [16384, 512] f32
    return full.reshape(BATCH, SEQ, DIN)
